# revision 31
# baseline (speedup 1.0000x reference)
"""DeepSeekMoE forward on 8 Trainium2 NeuronCores (Bass/Tile).

Strategy (expert-parallel, host dispatch/combine, mixed precision):
  - Router (sigmoid scores + top-4 + gating) on host jax-CPU, bitwise-matching
    the reference op sequence.
  - Per core 3 jobs: [routed expert A (big), shared-expert shard, routed
    expert B (small)]. Experts are paired big-with-small (sorted by token
    count) so per-slot capacities C1/C2 hug the actual max counts.
  - Shared experts: bf16 matmuls (1 col/cycle on the PE, same as fp32r, but
    half the DMA/SBUF). rel-err contribution ~2e-3.
  - Routed experts: fp8 e4m3 with DoubleRow perf mode: 2 weights/PE cell,
    K=256 per matmul instruction -> ~1.8x PE throughput. Activations and
    weights pre-scaled by powers of 2 (s_x=s_h=8, w scaled to std~16);
    scales undone in the activation stage (scale AP) and host gating.
    Measured end-to-end rel-err 1.31e-2 (gate 2e-2).
  - mm1: K=2048 = 8 DoubleRow steps; mm2: K=1408 = 5 DoubleRow + 1 plain
    fp8 step. Chunk-major inner loops keep each 256-col LDWEIGHTS hidden
    under >=2 chained 512-token matmuls.
  - Host scatters routed outputs with gating weights, adds residual+shared.
"""

import numpy as np
import ml_dtypes

D_MODEL, D_FF, NS, NR, KR = 2048, 1408, 2, 16, 4
P = 128
DT = D_MODEL // P   # 16
FT = D_FF // P      # 11
KP1 = D_MODEL // 256  # 8 DoubleRow steps in mm1
KP2 = D_FF // 256     # 5 DoubleRow steps in mm2 (+1 plain leftover)
NCORES = 8
SH_TOK = 2048
S_X = 8.0
S_H = 8.0

E4 = ml_dtypes.float8_e4m3
BF16 = ml_dtypes.bfloat16

_prog_cache = {}
LAST_RESULT = None


def _ensure_ntff_hook():
    """Install the axon NTFF profiler shim if antenv lacks axon_hooks
    (lets run_bass_kernel_spmd(trace=True) work under BASS_TRACE=1)."""
    try:
        from antenv.axon_hooks import get_axon_ntff_profile_hook  # noqa: F401
        return
    except ImportError:
        pass
    import sys
    import types
    try:
        import antenv
        mod = types.ModuleType("antenv.axon_hooks")
        _hook = [None]
        mod.set_axon_ntff_profile_hook = lambda h: _hook.__setitem__(0, h)
        mod.get_axon_ntff_profile_hook = lambda: _hook[0]
        sys.modules["antenv.axon_hooks"] = mod
        antenv.axon_hooks = mod
        from trn_agent_boot.trn_boot import _ntff_profile_via_ctypes
        mod.set_axon_ntff_profile_hook(
            _ntff_profile_via_ctypes("/opt/axon/libaxon_pjrt.so")
        )
    except Exception:
        pass


def _chunks(blk):
    out = [512] * (blk // 512)
    if blk % 512:
        out.append(blk % 512)
    return out


def _blocks(C):
    """1024-token blocks + a remainder block of <=1152 (minimal 512-chunk
    count). Guard against a tiny (<256) trailing block."""
    nb = max(1, -(-C // 1152))
    while C - 1024 * (nb - 1) > 1152:
        nb += 1
    out = [1024] * (nb - 1) + [C - 1024 * (nb - 1)]
    if nb >= 2 and out[-1] < 256:
        spill = out[-2] + out[-1]
        out[-2:] = [spill // 2, spill - spill // 2]
    return out


def _build_program(C1, C2):
    import concourse.mybir as mybir
    import concourse.tile as tile
    from concourse import bacc

    F32 = mybir.dt.float32
    BF = mybir.dt.bfloat16
    FP8 = mybir.dt.float8e4
    DR = mybir.MatmulPerfMode.DoubleRow
    Relu = mybir.ActivationFunctionType.Relu
    Identity = mybir.ActivationFunctionType.Identity

    # const-tile column layout: [b1s(11) b2s(16) b1a(11) b2a(16) b1b(11)
    #                            b2b(16) sc1a sc2a sc1b sc2b]
    CB1S, CB2S = 0, 11
    CB1A, CB2A = 27, 38
    CB1B, CB2B = 54, 65
    CSC = 81
    NCB = 85

    # job A leads with a small 512-token head block (own contiguous tensor,
    # one DMA) so the first matmul starts ~10us in instead of ~20us.
    HB = min(512, C1)
    blocksA = [HB] + (_blocks(C1 - HB) if C1 > HB else [])
    blocksB = _blocks(C2)
    NBA, BA = len(blocksA) - 1, max(blocksA[1:]) if len(blocksA) > 1 else 1
    NBB, BB = len(blocksB), max(blocksB)

    nc = bacc.Bacc(None, target_bir_lowering=False)
    # block-major x: xra[b, :, s] is contiguous per partition -> full-rate DMA
    xh = nc.dram_tensor("xh", [P, KP1, 2, HB], FP8, kind="ExternalInput")
    xra = nc.dram_tensor("xra", [NBA, P, KP1, 2, BA], FP8, kind="ExternalInput")
    xrb = nc.dram_tensor("xrb", [NBB, P, KP1, 2, BB], FP8, kind="ExternalInput")
    xs = nc.dram_tensor("xs", [P, DT, SH_TOK], BF, kind="ExternalInput")
    w1a = nc.dram_tensor("w1a", [FT, P, KP1, 2, P], FP8, kind="ExternalInput")
    w1b = nc.dram_tensor("w1b", [FT, P, KP1, 2, P], FP8, kind="ExternalInput")
    w2a = nc.dram_tensor("w2a", [DT, P, KP2, 2, P], FP8, kind="ExternalInput")
    w2b = nc.dram_tensor("w2b", [DT, P, KP2, 2, P], FP8, kind="ExternalInput")
    w2al = nc.dram_tensor("w2al", [DT, P, P], FP8, kind="ExternalInput")
    w2bl = nc.dram_tensor("w2bl", [DT, P, P], FP8, kind="ExternalInput")
    w1s = nc.dram_tensor("w1s", [FT, P, DT, P], BF, kind="ExternalInput")
    w2s = nc.dram_tensor("w2s", [DT, P, FT, P], BF, kind="ExternalInput")
    cb = nc.dram_tensor("cb", [P, NCB], F32, kind="ExternalInput")
    ya = nc.dram_tensor("ya", [DT, P, C1], BF, kind="ExternalOutput")
    yb = nc.dram_tensor("yb", [DT, P, C2], BF, kind="ExternalOutput")
    ys = nc.dram_tensor("ys", [DT, P, SH_TOK], BF, kind="ExternalOutput")

    with tile.TileContext(nc) as tc:
        with (
            tc.tile_pool(name="const", bufs=1) as const,
            tc.tile_pool(name="hr", bufs=2) as hrpool,
            tc.tile_pool(name="w1r", bufs=4) as w1rpool,
            tc.tile_pool(name="w2r", bufs=4) as w2rpool,
            tc.tile_pool(name="yr", bufs=2) as yrpool,
            tc.tile_pool(name="xs", bufs=1) as xspool,
            tc.tile_pool(name="hs", bufs=1) as hspool,
            tc.tile_pool(name="w1s", bufs=2) as w1spool,
            tc.tile_pool(name="w2s", bufs=2) as w2spool,
            tc.tile_pool(name="ps", bufs=8, space="PSUM") as pspool,
        ):
            cbt = const.tile([P, NCB], F32)
            nc.sync.dma_start(cbt[:], cb[:, :])

            def emit_routed_block(xap, w1d, w2d, w2ld, ydram, off, blk,
                                  cb1, cb2, csc1, csc2, w1_pre=None):
                """One token-block of a routed expert job (fp8 DoubleRow)."""
                chunks = _chunks(blk)
                h_t = hrpool.tile([P, FT, blk], FP8, tag="hr")
                for ft in range(FT):
                    if ft == 0 and w1_pre is not None:
                        w1_t = w1_pre
                    else:
                        w1_t = w1rpool.tile([P, KP1, 2, P], FP8, tag="w1r")
                        nc.sync.dma_start(w1_t[:], w1d[ft])
                    pss = [pspool.tile([P, 512], F32, tag="ps", name=f"ps{ci}")
                           for ci in range(len(chunks))]
                    for s in range(KP1):
                        coff = 0
                        for ci, ch in enumerate(chunks):
                            nc.tensor.matmul(
                                pss[ci][:, :ch],
                                w1_t[:, s],
                                xap(s, coff, ch),
                                start=(s == 0),
                                stop=(s == KP1 - 1),
                                perf_mode=DR,
                            )
                            coff += ch
                    coff = 0
                    for ci, ch in enumerate(chunks):
                        nc.scalar.activation(
                            h_t[:, ft, coff:coff + ch],
                            pss[ci][:, :ch],
                            Relu,
                            bias=cbt[:, cb1 + ft:cb1 + ft + 1],
                            scale=cbt[:, csc1:csc1 + 1],
                        )
                        coff += ch

                for dt_i in range(DT):
                    w2_t = w2rpool.tile([P, KP2, 2, P], FP8, tag="w2r")
                    nc.sync.dma_start(w2_t[:], w2d[dt_i])
                    w2l_t = w2rpool.tile([P, P], FP8, tag="w2l")
                    nc.sync.dma_start(w2l_t[:], w2ld[dt_i])
                    y_t = yrpool.tile([P, blk], BF, tag="yr")
                    pss = [pspool.tile([P, 512], F32, tag="ps", name=f"ps{ci}")
                           for ci in range(len(chunks))]
                    for s in range(KP2):
                        coff = 0
                        for ci, ch in enumerate(chunks):
                            nc.tensor.matmul(
                                pss[ci][:, :ch],
                                w2_t[:, s],
                                h_t[:, 2 * s:2 * s + 2, coff:coff + ch],
                                start=(s == 0),
                                stop=False,
                                perf_mode=DR,
                            )
                            coff += ch
                    coff = 0
                    for ci, ch in enumerate(chunks):
                        nc.tensor.matmul(
                            pss[ci][:, :ch],
                            w2l_t[:],
                            h_t[:, FT - 1, coff:coff + ch],
                            start=False,
                            stop=True,
                        )
                        coff += ch
                    coff = 0
                    for ci, ch in enumerate(chunks):
                        nc.scalar.activation(
                            y_t[:, coff:coff + ch],
                            pss[ci][:, :ch],
                            Identity,
                            bias=cbt[:, cb2 + dt_i:cb2 + dt_i + 1],
                            scale=cbt[:, csc2:csc2 + 1],
                        )
                        coff += ch
                    nc.scalar.dma_start(
                        ydram[dt_i, :, off:off + blk], y_t[:, :blk]
                    )

            def emit_routed_blocks(xrpool, xdram, w1d, w2d, w2ld, ydram,
                                   blocks, bmax, cb1, cb2, csc1, csc2,
                                   off0=0):
                off = off0
                for bi, blk in enumerate(blocks):
                    x_t = xrpool.tile([P, KP1, 2, bmax], FP8, tag="xr")
                    # per-kpair DMA slices on the SP HWDGE ring (the gpsimd
                    # SWDGE only issues ~1 DMA per 2.3us)
                    for s in range(KP1):
                        nc.sync.dma_start(x_t[:, s], xdram[bi, :, s])
                    xt_cur = x_t
                    emit_routed_block(
                        lambda s, c0, ch: xt_cur[:, s, :, c0:c0 + ch],
                        w1d, w2d, w2ld, ydram, off, blk,
                        cb1, cb2, csc1, csc2)
                    off += blk

            def emit_shared_job():
                chunks = _chunks(SH_TOK)   # [512]*4
                xs_t = xspool.tile([P, DT, SH_TOK], BF, tag="xs")
                for ko in range(DT):
                    eng = nc.gpsimd if ko % 2 == 0 else nc.scalar
                    eng.dma_start(xs_t[:, ko], xs[:, ko])
                h_t = hspool.tile([P, FT, SH_TOK], BF, tag="hs")
                for ft in range(FT):
                    w1_t = w1spool.tile([P, DT, P], BF, tag="w1s")
                    nc.sync.dma_start(w1_t[:], w1s[ft])
                    pss = [pspool.tile([P, 512], F32, tag="ps", name=f"ps{ci}")
                           for ci in range(len(chunks))]
                    for ko in range(DT):
                        coff = 0
                        for ci, ch in enumerate(chunks):
                            nc.tensor.matmul(
                                pss[ci][:, :ch],
                                w1_t[:, ko],
                                xs_t[:, ko, coff:coff + ch],
                                start=(ko == 0),
                                stop=(ko == DT - 1),
                            )
                            coff += ch
                    coff = 0
                    for ci, ch in enumerate(chunks):
                        nc.scalar.activation(
                            h_t[:, ft, coff:coff + ch],
                            pss[ci][:, :ch],
                            Relu,
                            bias=cbt[:, 0 + ft:0 + ft + 1],
                        )
                        coff += ch

                for dt_i in range(DT):
                    w2_t = w2spool.tile([P, FT, P], BF, tag="w2s")
                    nc.sync.dma_start(w2_t[:], w2s[dt_i])
                    y_t = yrpool.tile([P, SH_TOK], BF, tag="ysh", bufs=1)
                    pss = [pspool.tile([P, 512], F32, tag="ps", name=f"ps{ci}")
                           for ci in range(len(chunks))]
                    for ft in range(FT):
                        coff = 0
                        for ci, ch in enumerate(chunks):
                            nc.tensor.matmul(
                                pss[ci][:, :ch],
                                w2_t[:, ft],
                                h_t[:, ft, coff:coff + ch],
                                start=(ft == 0),
                                stop=(ft == FT - 1),
                            )
                            coff += ch
                    coff = 0
                    for ci, ch in enumerate(chunks):
                        nc.scalar.activation(
                            y_t[:, coff:coff + ch],
                            pss[ci][:, :ch],
                            Identity,
                            bias=cbt[:, 11 + dt_i:11 + dt_i + 1],
                        )
                        coff += ch
                    nc.scalar.dma_start(ys[dt_i, :, :], y_t[:, :SH_TOK])

            with tc.tile_pool(name="xr", bufs=2) as xrpool:
                # head block: w1[ft0] + one contiguous x DMA land first so
                # the first matmul issues ~10us in
                w1_pre = w1rpool.tile([P, KP1, 2, P], FP8, tag="w1r")
                nc.sync.dma_start(w1_pre[:], w1a[0])
                xh_t = xrpool.tile([P, KP1, 2, HB], FP8, tag="xh", bufs=1)
                nc.sync.dma_start(xh_t[:], xh[:, :, :, :])
                emit_routed_block(
                    lambda s, c0, ch: xh_t[:, s, :, c0:c0 + ch],
                    w1a, w2a, w2al, ya, 0, HB,
                    CB1A, CB2A, CSC, CSC + 1, w1_pre=w1_pre)
                if len(blocksA) > 1:
                    emit_routed_blocks(xrpool, xra, w1a, w2a, w2al, ya,
                                       blocksA[1:], BA,
                                       CB1A, CB2A, CSC, CSC + 1, off0=HB)
                emit_shared_job()
                emit_routed_blocks(xrpool, xrb, w1b, w2b, w2bl, yb,
                                   blocksB, BB, CB1B, CB2B, CSC + 2, CSC + 3)

    nc.compile()
    return nc


def _routing(flat, centroids, bias):
    """Replicate the reference router bitwise (jax-CPU sigmoid, stable
    top-4, normalized gates)."""
    import jax
    import jax.numpy as jnp

    cpu = jax.devices("cpu")[0]
    with jax.default_device(cpu):
        scores = np.asarray(
            jax.nn.sigmoid(jnp.asarray(flat) @ jnp.asarray(centroids).T)
            + jnp.asarray(bias)
        )
    idx = np.argsort(-scores, axis=-1, kind="stable")[:, :KR]
    vals = np.take_along_axis(scores, idx, axis=-1)
    gating = vals / np.maximum(vals.sum(-1, keepdims=True, dtype=np.float32), 1e-8)
    return idx.astype(np.int32), gating.astype(np.float32)


def _pow2_scale(w, target=16.0):
    s = float(w.astype(np.float64).std())
    return float(2.0 ** np.round(np.log2(target / max(s, 1e-30))))


def _q8(x, scale):
    return np.clip(x * np.float32(scale), -240.0, 240.0).astype(E4)


def _xr_interleave(x_td):
    """[cnt, D] tokens -> [P, KP1, 2, cnt] fp8 DoubleRow-interleaved, x*S_X."""
    xq = _q8(x_td.T, S_X)                         # [D, cnt]
    return xq.reshape(KP1, 2, P, -1).transpose(2, 0, 1, 3)


def _xr_pack(x_td, blocks):
    """[cnt, D] tokens -> block-major [NB, P, KP1, 2, BMAX] fp8."""
    nb, bmax = max(1, len(blocks)), max(blocks) if blocks else 1
    cnt = x_td.shape[0]
    out = np.zeros((nb, P, KP1, 2, bmax), dtype=E4)
    if cnt:
        xq = _xr_interleave(x_td)
        off = 0
        for bi, blk in enumerate(blocks):
            n = max(0, min(blk, cnt - off))
            if n:
                out[bi, :, :, :, :n] = xq[:, :, :, off:off + n]
            off += blk
    return out


def _xr_head(x_td, hb):
    """First hb tokens -> [P, KP1, 2, hb] fp8 (head block tensor)."""
    cnt = x_td.shape[0]
    out = np.zeros((P, KP1, 2, hb), dtype=E4)
    if cnt:
        n = min(hb, cnt)
        out[:, :, :, :n] = _xr_interleave(x_td[:n])
    return out


def _w1r_tiles(w, scale):
    """[D, F] -> [FT, P, KP1, 2, P] fp8 lhsT tiles (k = 256s+128i+p)."""
    wq = _q8(w, scale)
    return np.ascontiguousarray(
        wq.reshape(KP1, 2, P, FT, P).transpose(3, 2, 0, 1, 4)
    )


def _w2r_tiles(w, scale):
    """[F, D] -> pairs [DT, P, KP2, 2, P] + leftover [DT, P, P] fp8."""
    wq = _q8(w, scale)
    pairs = np.ascontiguousarray(
        wq[: KP2 * 256].reshape(KP2, 2, P, DT, P).transpose(3, 2, 0, 1, 4)
    )
    left = np.ascontiguousarray(
        wq[KP2 * 256:].reshape(P, DT, P).transpose(1, 0, 2)
    )
    return pairs, left


def _xs_tiles(x_td):
    """[T, D] -> [P, DT, T] bf16 feature-major."""
    return np.ascontiguousarray(
        x_td.T.reshape(DT, P, -1).transpose(1, 0, 2)
    ).astype(BF16)


def _w1s_tiles(w):
    """[D, F] -> [FT, P, DT, P] bf16 lhsT tiles."""
    return np.ascontiguousarray(
        w.astype(BF16).reshape(DT, P, FT, P).transpose(2, 1, 0, 3)
    )


def _w2s_tiles(w):
    """[F, D] -> [DT, P, FT, P] bf16 lhsT tiles."""
    return np.ascontiguousarray(
        w.astype(BF16).reshape(FT, P, DT, P).transpose(2, 1, 0, 3)
    )


def kernel(u, shared_w1, shared_b1, shared_w2, shared_b2,
           routed_w1, routed_b1, routed_w2, routed_b2, centroids, bias):
    from concourse.bass_utils import run_bass_kernel_spmd

    _ensure_ntff_hook()
    u = np.asarray(u, dtype=np.float32)
    b, s, d = u.shape
    flat = u.reshape(-1, d)
    T = flat.shape[0]

    idx, gating = _routing(flat, np.asarray(centroids, np.float32),
                           np.asarray(bias, np.float32))

    tok_lists, gate_lists = [], []
    counts = np.zeros(NR, dtype=np.int64)
    for e in range(NR):
        hit = idx == e
        rows = np.nonzero(hit.any(axis=1))[0]
        g = gating[hit].reshape(-1)
        tok_lists.append(rows)
        gate_lists.append(g.astype(np.float32))
        counts[e] = len(rows)

    # big-with-small expert pairing: slot A gets rank i, slot B rank 15-i
    order = np.argsort(-counts, kind="stable")
    slot_a = [int(order[i]) for i in range(NCORES)]
    slot_b = [int(order[NR - 1 - i]) for i in range(NCORES)]
    C1 = max(256, int(max(counts[e] for e in slot_a)))
    C2 = max(256, int(max(counts[e] for e in slot_b)))

    key = (C1, C2)
    if key not in _prog_cache:
        _prog_cache[key] = _build_program(C1, C2)
    nc = _prog_cache[key]

    sw1 = np.asarray(shared_w1, np.float32)
    sb1 = np.asarray(shared_b1, np.float32)
    sw2 = np.asarray(shared_w2, np.float32)
    sb2 = np.asarray(shared_b2, np.float32)
    rw1 = np.asarray(routed_w1, np.float32)
    rb1 = np.asarray(routed_b1, np.float32)
    rw2 = np.asarray(routed_w2, np.float32)
    rb2 = np.asarray(routed_b2, np.float32)

    s_w1 = [_pow2_scale(rw1[e]) for e in range(NR)]
    s_w2 = [_pow2_scale(rw2[e]) for e in range(NR)]
    rw1_t = [_w1r_tiles(rw1[e], s_w1[e]) for e in range(NR)]
    rw2_t = [_w2r_tiles(rw2[e], s_w2[e]) for e in range(NR)]
    sw1_t = [_w1s_tiles(sw1[n]) for n in range(NS)]
    sw2_t = [_w2s_tiles(sw2[n]) for n in range(NS)]

    in_maps = []
    for core in range(NCORES):
        sh_e = core % NS
        sh_off = (core // NS) * SH_TOK
        ea, eb = slot_a[core], slot_b[core]

        cbm = np.zeros((P, 85), np.float32)
        cbm[:, 0:11] = sb1[sh_e].reshape(FT, P).T
        cbm[:, 11:27] = sb2[sh_e].reshape(DT, P).T
        cbm[:, 27:38] = rb1[ea].reshape(FT, P).T * S_H
        cbm[:, 38:54] = rb2[ea].reshape(DT, P).T
        cbm[:, 54:65] = rb1[eb].reshape(FT, P).T * S_H
        cbm[:, 65:81] = rb2[eb].reshape(DT, P).T
        cbm[:, 81] = 1.0 / s_w1[ea]
        cbm[:, 82] = 1.0 / (S_H * s_w2[ea])
        cbm[:, 83] = 1.0 / s_w1[eb]
        cbm[:, 84] = 1.0 / (S_H * s_w2[eb])

        w2a_p, w2a_l = rw2_t[ea]
        w2b_p, w2b_l = rw2_t[eb]
        hb = min(512, C1)
        xa = flat[tok_lists[ea]]
        in_maps.append({
            "xh": _xr_head(xa, hb),
            "xra": _xr_pack(xa[hb:], _blocks(C1 - hb) if C1 > hb else []),
            "xrb": _xr_pack(flat[tok_lists[eb]], _blocks(C2)),
            "xs": _xs_tiles(flat[sh_off:sh_off + SH_TOK]),
            "w1a": rw1_t[ea], "w1b": rw1_t[eb],
            "w2a": w2a_p, "w2al": w2a_l,
            "w2b": w2b_p, "w2bl": w2b_l,
            "w1s": sw1_t[sh_e], "w2s": sw2_t[sh_e],
            "cb": cbm,
        })

    res = run_bass_kernel_spmd(nc, in_maps, core_ids=list(range(NCORES)))
    global LAST_RESULT
    LAST_RESULT = res

    out = flat.copy()
    for core in range(NCORES):
        r = res.results[core]
        sh_off = (core // NS) * SH_TOK
        out[sh_off:sh_off + SH_TOK] += (
            r["ys"].astype(np.float32).reshape(D_MODEL, SH_TOK).T
        )
        for nm, e, C in (("ya", slot_a[core], C1), ("yb", slot_b[core], C2)):
            rows = tok_lists[e]
            if len(rows):
                ye = (r[nm].astype(np.float32)
                      .reshape(D_MODEL, C)[:, :len(rows)].T)
                out[rows] += gate_lists[e][:, None] * ye

    return out.reshape(b, s, d)


# revision 33
# speedup vs baseline: 1.0209x; 1.0209x over previous
"""DeepSeekMoE forward on 8 Trainium2 NeuronCores (Bass/Tile).

Strategy (expert-parallel, host dispatch/combine, mixed precision):
  - Router (sigmoid scores + top-4 + gating) on host jax-CPU, bitwise-matching
    the reference op sequence.
  - Per core 3 jobs: [routed expert A (big), shared-expert shard, routed
    expert B (small)]. Experts are paired big-with-small (sorted by token
    count) so per-slot capacities C1/C2 hug the actual max counts.
  - Shared experts: bf16 matmuls (1 col/cycle on the PE, same as fp32r, but
    half the DMA/SBUF). rel-err contribution ~2e-3.
  - Routed experts: fp8 e4m3 with DoubleRow perf mode: 2 weights/PE cell,
    K=256 per matmul instruction -> ~1.8x PE throughput. Activations and
    weights pre-scaled by powers of 2 (s_x=s_h=8, w scaled to std~16);
    scales undone in the activation stage (scale AP) and host gating.
    Measured end-to-end rel-err 1.31e-2 (gate 2e-2).
  - mm1: K=2048 = 8 DoubleRow steps; mm2: K=1408 = 5 DoubleRow + 1 plain
    fp8 step. Chunk-major inner loops keep each 256-col LDWEIGHTS hidden
    under >=2 chained 512-token matmuls.
  - Host scatters routed outputs with gating weights, adds residual+shared.
"""

import numpy as np
import ml_dtypes

D_MODEL, D_FF, NS, NR, KR = 2048, 1408, 2, 16, 4
P = 128
DT = D_MODEL // P   # 16
FT = D_FF // P      # 11
KP1 = D_MODEL // 256  # 8 DoubleRow steps in mm1
KP2 = D_FF // 256     # 5 DoubleRow steps in mm2 (+1 plain leftover)
NCORES = 8
SH_TOK = 2048
S_X = 8.0
S_H = 8.0

E4 = ml_dtypes.float8_e4m3
BF16 = ml_dtypes.bfloat16

_prog_cache = {}
LAST_RESULT = None


def _ensure_ntff_hook():
    """Install the axon NTFF profiler shim if antenv lacks axon_hooks
    (lets run_bass_kernel_spmd(trace=True) work under BASS_TRACE=1)."""
    try:
        from antenv.axon_hooks import get_axon_ntff_profile_hook  # noqa: F401
        return
    except ImportError:
        pass
    import sys
    import types
    try:
        import antenv
        mod = types.ModuleType("antenv.axon_hooks")
        _hook = [None]
        mod.set_axon_ntff_profile_hook = lambda h: _hook.__setitem__(0, h)
        mod.get_axon_ntff_profile_hook = lambda: _hook[0]
        sys.modules["antenv.axon_hooks"] = mod
        antenv.axon_hooks = mod
        from trn_agent_boot.trn_boot import _ntff_profile_via_ctypes
        mod.set_axon_ntff_profile_hook(
            _ntff_profile_via_ctypes("/opt/axon/libaxon_pjrt.so")
        )
    except Exception:
        pass


def _chunks(blk):
    out = [512] * (blk // 512)
    if blk % 512:
        out.append(blk % 512)
    return out


def _blocks(C):
    """1024-token blocks + a remainder block of <=1152 (minimal 512-chunk
    count). Guard against a tiny (<256) trailing block."""
    nb = max(1, -(-C // 1152))
    while C - 1024 * (nb - 1) > 1152:
        nb += 1
    out = [1024] * (nb - 1) + [C - 1024 * (nb - 1)]
    if nb >= 2 and out[-1] < 256:
        spill = out[-2] + out[-1]
        out[-2:] = [spill // 2, spill - spill // 2]
    return out


def _build_program(C1, C2):
    import concourse.mybir as mybir
    import concourse.tile as tile
    from concourse import bacc

    F32 = mybir.dt.float32
    BF = mybir.dt.bfloat16
    FP8 = mybir.dt.float8e4
    DR = mybir.MatmulPerfMode.DoubleRow
    Relu = mybir.ActivationFunctionType.Relu
    Identity = mybir.ActivationFunctionType.Identity

    # const-tile column layout: [b1s(11) b2s(16) b1a(11) b2a(16) b1b(11)
    #                            b2b(16) sc1a sc2a sc1b sc2b]
    CB1S, CB2S = 0, 11
    CB1A, CB2A = 27, 38
    CB1B, CB2B = 54, 65
    CSC = 81
    NCB = 85

    blocksA, blocksB = _blocks(C1), _blocks(C2)
    NBA, BA = len(blocksA), max(blocksA)
    NBB, BB = len(blocksB), max(blocksB)

    nc = bacc.Bacc(None, target_bir_lowering=False)
    # block-major x: xra[b, :, s] is contiguous per partition -> full-rate DMA
    xra = nc.dram_tensor("xra", [NBA, P, KP1, 2, BA], FP8, kind="ExternalInput")
    xrb = nc.dram_tensor("xrb", [NBB, P, KP1, 2, BB], FP8, kind="ExternalInput")
    xs = nc.dram_tensor("xs", [P, DT, SH_TOK], BF, kind="ExternalInput")
    w1a = nc.dram_tensor("w1a", [FT, P, KP1, 2, P], FP8, kind="ExternalInput")
    w1b = nc.dram_tensor("w1b", [FT, P, KP1, 2, P], FP8, kind="ExternalInput")
    w2a = nc.dram_tensor("w2a", [DT, P, KP2, 2, P], FP8, kind="ExternalInput")
    w2b = nc.dram_tensor("w2b", [DT, P, KP2, 2, P], FP8, kind="ExternalInput")
    w2al = nc.dram_tensor("w2al", [DT, P, P], FP8, kind="ExternalInput")
    w2bl = nc.dram_tensor("w2bl", [DT, P, P], FP8, kind="ExternalInput")
    w1s = nc.dram_tensor("w1s", [FT, P, DT, P], BF, kind="ExternalInput")
    w2s = nc.dram_tensor("w2s", [DT, P, FT, P], BF, kind="ExternalInput")
    cb = nc.dram_tensor("cb", [P, NCB], F32, kind="ExternalInput")
    ya = nc.dram_tensor("ya", [DT, P, C1], BF, kind="ExternalOutput")
    yb = nc.dram_tensor("yb", [DT, P, C2], BF, kind="ExternalOutput")
    ys = nc.dram_tensor("ys", [DT, P, SH_TOK], BF, kind="ExternalOutput")

    with tile.TileContext(nc) as tc:
        with (
            tc.tile_pool(name="const", bufs=1) as const,
            tc.tile_pool(name="hr", bufs=2) as hrpool,
            tc.tile_pool(name="w1r", bufs=4) as w1rpool,
            tc.tile_pool(name="w2r", bufs=4) as w2rpool,
            tc.tile_pool(name="yr", bufs=2) as yrpool,
            tc.tile_pool(name="xs", bufs=1) as xspool,
            tc.tile_pool(name="hs", bufs=1) as hspool,
            tc.tile_pool(name="w1s", bufs=2) as w1spool,
            tc.tile_pool(name="w2s", bufs=2) as w2spool,
            tc.tile_pool(name="ps", bufs=8, space="PSUM") as pspool,
        ):
            cbt = const.tile([P, NCB], F32)
            nc.sync.dma_start(cbt[:], cb[:, :])

            # ~20 throwaway matmuls on the const tile warm the PE clock
            # (HAM 4/8 -> 8/8 needs ~3.4us of sustained activity) while the
            # first x block is still in flight
            for wi in range(20):
                wps = pspool.tile([P, NCB], F32, tag="ps", name=f"warm{wi}")
                nc.tensor.matmul(
                    wps[0:NCB, 0:NCB],
                    cbt[:, 0:NCB],
                    cbt[:, 0:NCB],
                    start=True,
                    stop=True,
                )

            def emit_routed_block(xap, w1d, w2d, w2ld, ydram, off, blk,
                                  cb1, cb2, csc1, csc2, w1_pre=None):
                """One token-block of a routed expert job (fp8 DoubleRow)."""
                chunks = _chunks(blk)
                h_t = hrpool.tile([P, FT, blk], FP8, tag="hr")
                for ft in range(FT):
                    if ft == 0 and w1_pre is not None:
                        w1_t = w1_pre
                    else:
                        w1_t = w1rpool.tile([P, KP1, 2, P], FP8, tag="w1r")
                        nc.sync.dma_start(w1_t[:], w1d[ft])
                    pss = [pspool.tile([P, 512], F32, tag="ps", name=f"ps{ci}")
                           for ci in range(len(chunks))]
                    for s in range(KP1):
                        coff = 0
                        for ci, ch in enumerate(chunks):
                            nc.tensor.matmul(
                                pss[ci][:, :ch],
                                w1_t[:, s],
                                xap(s, coff, ch),
                                start=(s == 0),
                                stop=(s == KP1 - 1),
                                perf_mode=DR,
                            )
                            coff += ch
                    coff = 0
                    for ci, ch in enumerate(chunks):
                        nc.scalar.activation(
                            h_t[:, ft, coff:coff + ch],
                            pss[ci][:, :ch],
                            Relu,
                            bias=cbt[:, cb1 + ft:cb1 + ft + 1],
                            scale=cbt[:, csc1:csc1 + 1],
                        )
                        coff += ch

                for dt_i in range(DT):
                    w2_t = w2rpool.tile([P, KP2, 2, P], FP8, tag="w2r")
                    nc.sync.dma_start(w2_t[:], w2d[dt_i])
                    w2l_t = w2rpool.tile([P, P], FP8, tag="w2l")
                    nc.sync.dma_start(w2l_t[:], w2ld[dt_i])
                    y_t = yrpool.tile([P, blk], BF, tag="yr")
                    pss = [pspool.tile([P, 512], F32, tag="ps", name=f"ps{ci}")
                           for ci in range(len(chunks))]
                    for s in range(KP2):
                        coff = 0
                        for ci, ch in enumerate(chunks):
                            nc.tensor.matmul(
                                pss[ci][:, :ch],
                                w2_t[:, s],
                                h_t[:, 2 * s:2 * s + 2, coff:coff + ch],
                                start=(s == 0),
                                stop=False,
                                perf_mode=DR,
                            )
                            coff += ch
                    coff = 0
                    for ci, ch in enumerate(chunks):
                        nc.tensor.matmul(
                            pss[ci][:, :ch],
                            w2l_t[:],
                            h_t[:, FT - 1, coff:coff + ch],
                            start=False,
                            stop=True,
                        )
                        coff += ch
                    coff = 0
                    for ci, ch in enumerate(chunks):
                        nc.scalar.activation(
                            y_t[:, coff:coff + ch],
                            pss[ci][:, :ch],
                            Identity,
                            bias=cbt[:, cb2 + dt_i:cb2 + dt_i + 1],
                            scale=cbt[:, csc2:csc2 + 1],
                        )
                        coff += ch
                    nc.scalar.dma_start(
                        ydram[dt_i, :, off:off + blk], y_t[:, :blk]
                    )

            def emit_routed_blocks(xrpool, xdram, w1d, w2d, w2ld, ydram,
                                   blocks, bmax, cb1, cb2, csc1, csc2,
                                   head=False):
                off = 0
                for bi, blk in enumerate(blocks):
                    w1_first = None
                    if bi == 0 and head:
                        w1_first = [w1rpool.tile([P, KP1, 2, P], FP8,
                                                 tag="w1r", name="w1f")]
                    x_t = xrpool.tile([P, KP1, 2, bmax], FP8, tag="xr")
                    # one contiguous DMA on the SP HWDGE ring (deps are
                    # tile-granular, so finer slicing buys nothing and the
                    # extra issues stall the ring at the kernel head)
                    if w1_first is not None:
                        nc.sync.dma_start(w1_first[0][:], w1d[0])
                    nc.sync.dma_start(x_t[:], xdram[bi])
                    xt_cur = x_t
                    emit_routed_block(
                        lambda s, c0, ch: xt_cur[:, s, :, c0:c0 + ch],
                        w1d, w2d, w2ld, ydram, off, blk,
                        cb1, cb2, csc1, csc2,
                        w1_pre=w1_first[0] if w1_first else None)
                    off += blk

            def emit_shared_job():
                chunks = _chunks(SH_TOK)   # [512]*4
                xs_t = xspool.tile([P, DT, SH_TOK], BF, tag="xs")
                for ko in range(DT):
                    eng = nc.gpsimd if ko % 2 == 0 else nc.scalar
                    eng.dma_start(xs_t[:, ko], xs[:, ko])
                h_t = hspool.tile([P, FT, SH_TOK], BF, tag="hs")
                for ft in range(FT):
                    w1_t = w1spool.tile([P, DT, P], BF, tag="w1s")
                    nc.sync.dma_start(w1_t[:], w1s[ft])
                    pss = [pspool.tile([P, 512], F32, tag="ps", name=f"ps{ci}")
                           for ci in range(len(chunks))]
                    for ko in range(DT):
                        coff = 0
                        for ci, ch in enumerate(chunks):
                            nc.tensor.matmul(
                                pss[ci][:, :ch],
                                w1_t[:, ko],
                                xs_t[:, ko, coff:coff + ch],
                                start=(ko == 0),
                                stop=(ko == DT - 1),
                            )
                            coff += ch
                    coff = 0
                    for ci, ch in enumerate(chunks):
                        nc.scalar.activation(
                            h_t[:, ft, coff:coff + ch],
                            pss[ci][:, :ch],
                            Relu,
                            bias=cbt[:, 0 + ft:0 + ft + 1],
                        )
                        coff += ch

                for dt_i in range(DT):
                    w2_t = w2spool.tile([P, FT, P], BF, tag="w2s")
                    nc.sync.dma_start(w2_t[:], w2s[dt_i])
                    y_t = yrpool.tile([P, SH_TOK], BF, tag="ysh", bufs=1)
                    pss = [pspool.tile([P, 512], F32, tag="ps", name=f"ps{ci}")
                           for ci in range(len(chunks))]
                    for ft in range(FT):
                        coff = 0
                        for ci, ch in enumerate(chunks):
                            nc.tensor.matmul(
                                pss[ci][:, :ch],
                                w2_t[:, ft],
                                h_t[:, ft, coff:coff + ch],
                                start=(ft == 0),
                                stop=(ft == FT - 1),
                            )
                            coff += ch
                    coff = 0
                    for ci, ch in enumerate(chunks):
                        nc.scalar.activation(
                            y_t[:, coff:coff + ch],
                            pss[ci][:, :ch],
                            Identity,
                            bias=cbt[:, 11 + dt_i:11 + dt_i + 1],
                        )
                        coff += ch
                    nc.scalar.dma_start(ys[dt_i, :, :], y_t[:, :SH_TOK])

            with tc.tile_pool(name="xr", bufs=2) as xrpool:
                emit_routed_blocks(xrpool, xra, w1a, w2a, w2al, ya,
                                   blocksA, BA, CB1A, CB2A, CSC, CSC + 1,
                                   head=True)
                emit_shared_job()
                emit_routed_blocks(xrpool, xrb, w1b, w2b, w2bl, yb,
                                   blocksB, BB, CB1B, CB2B, CSC + 2, CSC + 3)

    nc.compile()
    return nc


def _routing(flat, centroids, bias):
    """Replicate the reference router bitwise (jax-CPU sigmoid, stable
    top-4, normalized gates)."""
    import jax
    import jax.numpy as jnp

    cpu = jax.devices("cpu")[0]
    with jax.default_device(cpu):
        scores = np.asarray(
            jax.nn.sigmoid(jnp.asarray(flat) @ jnp.asarray(centroids).T)
            + jnp.asarray(bias)
        )
    idx = np.argsort(-scores, axis=-1, kind="stable")[:, :KR]
    vals = np.take_along_axis(scores, idx, axis=-1)
    gating = vals / np.maximum(vals.sum(-1, keepdims=True, dtype=np.float32), 1e-8)
    return idx.astype(np.int32), gating.astype(np.float32)


def _pow2_scale(w, target=16.0):
    s = float(w.astype(np.float64).std())
    return float(2.0 ** np.round(np.log2(target / max(s, 1e-30))))


def _q8(x, scale):
    return np.clip(x * np.float32(scale), -240.0, 240.0).astype(E4)


def _xr_interleave(x_td):
    """[cnt, D] tokens -> [P, KP1, 2, cnt] fp8 DoubleRow-interleaved, x*S_X."""
    xq = _q8(x_td.T, S_X)                         # [D, cnt]
    return xq.reshape(KP1, 2, P, -1).transpose(2, 0, 1, 3)


def _xr_pack(x_td, blocks):
    """[cnt, D] tokens -> block-major [NB, P, KP1, 2, BMAX] fp8."""
    nb, bmax = max(1, len(blocks)), max(blocks) if blocks else 1
    cnt = x_td.shape[0]
    out = np.zeros((nb, P, KP1, 2, bmax), dtype=E4)
    if cnt:
        xq = _xr_interleave(x_td)
        off = 0
        for bi, blk in enumerate(blocks):
            n = max(0, min(blk, cnt - off))
            if n:
                out[bi, :, :, :, :n] = xq[:, :, :, off:off + n]
            off += blk
    return out


def _xr_head(x_td, hb):
    """First hb tokens -> [P, KP1, 2, hb] fp8 (head block tensor)."""
    cnt = x_td.shape[0]
    out = np.zeros((P, KP1, 2, hb), dtype=E4)
    if cnt:
        n = min(hb, cnt)
        out[:, :, :, :n] = _xr_interleave(x_td[:n])
    return out


def _w1r_tiles(w, scale):
    """[D, F] -> [FT, P, KP1, 2, P] fp8 lhsT tiles (k = 256s+128i+p)."""
    wq = _q8(w, scale)
    return np.ascontiguousarray(
        wq.reshape(KP1, 2, P, FT, P).transpose(3, 2, 0, 1, 4)
    )


def _w2r_tiles(w, scale):
    """[F, D] -> pairs [DT, P, KP2, 2, P] + leftover [DT, P, P] fp8."""
    wq = _q8(w, scale)
    pairs = np.ascontiguousarray(
        wq[: KP2 * 256].reshape(KP2, 2, P, DT, P).transpose(3, 2, 0, 1, 4)
    )
    left = np.ascontiguousarray(
        wq[KP2 * 256:].reshape(P, DT, P).transpose(1, 0, 2)
    )
    return pairs, left


def _xs_tiles(x_td):
    """[T, D] -> [P, DT, T] bf16 feature-major."""
    return np.ascontiguousarray(
        x_td.T.reshape(DT, P, -1).transpose(1, 0, 2)
    ).astype(BF16)


def _w1s_tiles(w):
    """[D, F] -> [FT, P, DT, P] bf16 lhsT tiles."""
    return np.ascontiguousarray(
        w.astype(BF16).reshape(DT, P, FT, P).transpose(2, 1, 0, 3)
    )


def _w2s_tiles(w):
    """[F, D] -> [DT, P, FT, P] bf16 lhsT tiles."""
    return np.ascontiguousarray(
        w.astype(BF16).reshape(FT, P, DT, P).transpose(2, 1, 0, 3)
    )


def kernel(u, shared_w1, shared_b1, shared_w2, shared_b2,
           routed_w1, routed_b1, routed_w2, routed_b2, centroids, bias):
    from concourse.bass_utils import run_bass_kernel_spmd

    _ensure_ntff_hook()
    u = np.asarray(u, dtype=np.float32)
    b, s, d = u.shape
    flat = u.reshape(-1, d)
    T = flat.shape[0]

    idx, gating = _routing(flat, np.asarray(centroids, np.float32),
                           np.asarray(bias, np.float32))

    tok_lists, gate_lists = [], []
    counts = np.zeros(NR, dtype=np.int64)
    for e in range(NR):
        hit = idx == e
        rows = np.nonzero(hit.any(axis=1))[0]
        g = gating[hit].reshape(-1)
        tok_lists.append(rows)
        gate_lists.append(g.astype(np.float32))
        counts[e] = len(rows)

    # big-with-small expert pairing: slot A gets rank i, slot B rank 15-i
    order = np.argsort(-counts, kind="stable")
    slot_a = [int(order[i]) for i in range(NCORES)]
    slot_b = [int(order[NR - 1 - i]) for i in range(NCORES)]
    C1 = max(256, int(max(counts[e] for e in slot_a)))
    C2 = max(256, int(max(counts[e] for e in slot_b)))

    key = (C1, C2)
    if key not in _prog_cache:
        _prog_cache[key] = _build_program(C1, C2)
    nc = _prog_cache[key]

    sw1 = np.asarray(shared_w1, np.float32)
    sb1 = np.asarray(shared_b1, np.float32)
    sw2 = np.asarray(shared_w2, np.float32)
    sb2 = np.asarray(shared_b2, np.float32)
    rw1 = np.asarray(routed_w1, np.float32)
    rb1 = np.asarray(routed_b1, np.float32)
    rw2 = np.asarray(routed_w2, np.float32)
    rb2 = np.asarray(routed_b2, np.float32)

    s_w1 = [_pow2_scale(rw1[e]) for e in range(NR)]
    s_w2 = [_pow2_scale(rw2[e]) for e in range(NR)]
    rw1_t = [_w1r_tiles(rw1[e], s_w1[e]) for e in range(NR)]
    rw2_t = [_w2r_tiles(rw2[e], s_w2[e]) for e in range(NR)]
    sw1_t = [_w1s_tiles(sw1[n]) for n in range(NS)]
    sw2_t = [_w2s_tiles(sw2[n]) for n in range(NS)]

    in_maps = []
    for core in range(NCORES):
        sh_e = core % NS
        sh_off = (core // NS) * SH_TOK
        ea, eb = slot_a[core], slot_b[core]

        cbm = np.zeros((P, 85), np.float32)
        cbm[:, 0:11] = sb1[sh_e].reshape(FT, P).T
        cbm[:, 11:27] = sb2[sh_e].reshape(DT, P).T
        cbm[:, 27:38] = rb1[ea].reshape(FT, P).T * S_H
        cbm[:, 38:54] = rb2[ea].reshape(DT, P).T
        cbm[:, 54:65] = rb1[eb].reshape(FT, P).T * S_H
        cbm[:, 65:81] = rb2[eb].reshape(DT, P).T
        cbm[:, 81] = 1.0 / s_w1[ea]
        cbm[:, 82] = 1.0 / (S_H * s_w2[ea])
        cbm[:, 83] = 1.0 / s_w1[eb]
        cbm[:, 84] = 1.0 / (S_H * s_w2[eb])

        w2a_p, w2a_l = rw2_t[ea]
        w2b_p, w2b_l = rw2_t[eb]
        in_maps.append({
            "xra": _xr_pack(flat[tok_lists[ea]], _blocks(C1)),
            "xrb": _xr_pack(flat[tok_lists[eb]], _blocks(C2)),
            "xs": _xs_tiles(flat[sh_off:sh_off + SH_TOK]),
            "w1a": rw1_t[ea], "w1b": rw1_t[eb],
            "w2a": w2a_p, "w2al": w2a_l,
            "w2b": w2b_p, "w2bl": w2b_l,
            "w1s": sw1_t[sh_e], "w2s": sw2_t[sh_e],
            "cb": cbm,
        })

    res = run_bass_kernel_spmd(nc, in_maps, core_ids=list(range(NCORES)))
    global LAST_RESULT
    LAST_RESULT = res

    out = flat.copy()
    for core in range(NCORES):
        r = res.results[core]
        sh_off = (core // NS) * SH_TOK
        out[sh_off:sh_off + SH_TOK] += (
            r["ys"].astype(np.float32).reshape(D_MODEL, SH_TOK).T
        )
        for nm, e, C in (("ya", slot_a[core], C1), ("yb", slot_b[core], C2)):
            rows = tok_lists[e]
            if len(rows):
                ye = (r[nm].astype(np.float32)
                      .reshape(D_MODEL, C)[:, :len(rows)].T)
                out[rows] += gate_lists[e][:, None] * ye

    return out.reshape(b, s, d)


# revision 34
# speedup vs baseline: 1.0302x; 1.0091x over previous
"""DeepSeekMoE forward on 8 Trainium2 NeuronCores (Bass/Tile).

Strategy (expert-parallel, host dispatch/combine, mixed precision):
  - Router (sigmoid scores + top-4 + gating) on host jax-CPU, bitwise-matching
    the reference op sequence.
  - Per core 3 jobs: [routed expert A (big), shared-expert shard, routed
    expert B (small)]. Experts are paired big-with-small (sorted by token
    count) so per-slot capacities C1/C2 hug the actual max counts.
  - Shared experts: bf16 matmuls (1 col/cycle on the PE, same as fp32r, but
    half the DMA/SBUF). rel-err contribution ~2e-3.
  - Routed experts: fp8 e4m3 with DoubleRow perf mode: 2 weights/PE cell,
    K=256 per matmul instruction -> ~1.8x PE throughput. Activations and
    weights pre-scaled by powers of 2 (s_x=s_h=8, w scaled to std~16);
    scales undone in the activation stage (scale AP) and host gating.
    Measured end-to-end rel-err 1.31e-2 (gate 2e-2).
  - mm1: K=2048 = 8 DoubleRow steps; mm2: K=1408 = 5 DoubleRow + 1 plain
    fp8 step. Chunk-major inner loops keep each 256-col LDWEIGHTS hidden
    under >=2 chained 512-token matmuls.
  - Host scatters routed outputs with gating weights, adds residual+shared.
"""

import numpy as np
import ml_dtypes

D_MODEL, D_FF, NS, NR, KR = 2048, 1408, 2, 16, 4
P = 128
DT = D_MODEL // P   # 16
FT = D_FF // P      # 11
KP1 = D_MODEL // 256  # 8 DoubleRow steps in mm1
KP2 = D_FF // 256     # 5 DoubleRow steps in mm2 (+1 plain leftover)
NCORES = 8
SH_TOK = 2048
S_X = 8.0
S_H = 8.0

E4 = ml_dtypes.float8_e4m3
BF16 = ml_dtypes.bfloat16

_prog_cache = {}
LAST_RESULT = None


def _ensure_ntff_hook():
    """Install the axon NTFF profiler shim if antenv lacks axon_hooks
    (lets run_bass_kernel_spmd(trace=True) work under BASS_TRACE=1)."""
    try:
        from antenv.axon_hooks import get_axon_ntff_profile_hook  # noqa: F401
        return
    except ImportError:
        pass
    import sys
    import types
    try:
        import antenv
        mod = types.ModuleType("antenv.axon_hooks")
        _hook = [None]
        mod.set_axon_ntff_profile_hook = lambda h: _hook.__setitem__(0, h)
        mod.get_axon_ntff_profile_hook = lambda: _hook[0]
        sys.modules["antenv.axon_hooks"] = mod
        antenv.axon_hooks = mod
        from trn_agent_boot.trn_boot import _ntff_profile_via_ctypes
        mod.set_axon_ntff_profile_hook(
            _ntff_profile_via_ctypes("/opt/axon/libaxon_pjrt.so")
        )
    except Exception:
        pass


def _chunks(blk):
    out = [512] * (blk // 512)
    if blk % 512:
        out.append(blk % 512)
    return out


def _blocks(C):
    """1024-token blocks + a remainder block of <=1152 (minimal 512-chunk
    count). Guard against a tiny (<256) trailing block."""
    nb = max(1, -(-C // 1152))
    while C - 1024 * (nb - 1) > 1152:
        nb += 1
    out = [1024] * (nb - 1) + [C - 1024 * (nb - 1)]
    if nb >= 2 and out[-1] < 256:
        spill = out[-2] + out[-1]
        out[-2:] = [spill // 2, spill - spill // 2]
    return out


def _build_program(C1, C2):
    import concourse.mybir as mybir
    import concourse.tile as tile
    from concourse import bacc

    F32 = mybir.dt.float32
    BF = mybir.dt.bfloat16
    FP8 = mybir.dt.float8e4
    DR = mybir.MatmulPerfMode.DoubleRow
    Relu = mybir.ActivationFunctionType.Relu
    Identity = mybir.ActivationFunctionType.Identity

    # const-tile column layout: [b1s(11) b2s(16) b1a(11) b2a(16) b1b(11)
    #                            b2b(16) sc1a sc2a sc1b sc2b]
    CB1S, CB2S = 0, 11
    CB1A, CB2A = 27, 38
    CB1B, CB2B = 54, 65
    CSC = 81
    NCB = 85

    blocksA, blocksB = _blocks(C1), _blocks(C2)
    NBA, BA = len(blocksA), max(blocksA)
    NBB, BB = len(blocksB), max(blocksB)

    nc = bacc.Bacc(None, target_bir_lowering=False)
    # block-major x: xra[b, :, s] is contiguous per partition -> full-rate DMA
    xra = nc.dram_tensor("xra", [NBA, P, KP1, 2, BA], FP8, kind="ExternalInput")
    xrb = nc.dram_tensor("xrb", [NBB, P, KP1, 2, BB], FP8, kind="ExternalInput")
    xs = nc.dram_tensor("xs", [P, DT, SH_TOK], BF, kind="ExternalInput")
    w1a = nc.dram_tensor("w1a", [FT, P, KP1, 2, P], FP8, kind="ExternalInput")
    w1b = nc.dram_tensor("w1b", [FT, P, KP1, 2, P], FP8, kind="ExternalInput")
    w2a = nc.dram_tensor("w2a", [DT, P, KP2, 2, P], FP8, kind="ExternalInput")
    w2b = nc.dram_tensor("w2b", [DT, P, KP2, 2, P], FP8, kind="ExternalInput")
    w2al = nc.dram_tensor("w2al", [DT, P, P], FP8, kind="ExternalInput")
    w2bl = nc.dram_tensor("w2bl", [DT, P, P], FP8, kind="ExternalInput")
    w1s = nc.dram_tensor("w1s", [FT, P, DT, P], BF, kind="ExternalInput")
    w2s = nc.dram_tensor("w2s", [DT, P, FT, P], BF, kind="ExternalInput")
    cb = nc.dram_tensor("cb", [P, NCB], F32, kind="ExternalInput")
    ya = nc.dram_tensor("ya", [DT, P, C1], BF, kind="ExternalOutput")
    yb = nc.dram_tensor("yb", [DT, P, C2], BF, kind="ExternalOutput")
    ys = nc.dram_tensor("ys", [DT, P, SH_TOK], BF, kind="ExternalOutput")

    with tile.TileContext(nc) as tc:
        with (
            tc.tile_pool(name="const", bufs=1) as const,
            tc.tile_pool(name="hr", bufs=2) as hrpool,
            tc.tile_pool(name="w1r", bufs=4) as w1rpool,
            tc.tile_pool(name="w2r", bufs=4) as w2rpool,
            tc.tile_pool(name="yr", bufs=2) as yrpool,
            tc.tile_pool(name="xs", bufs=1) as xspool,
            tc.tile_pool(name="hs", bufs=1) as hspool,
            tc.tile_pool(name="w1s", bufs=2) as w1spool,
            tc.tile_pool(name="w2s", bufs=2) as w2spool,
            tc.tile_pool(name="ps", bufs=8, space="PSUM") as pspool,
        ):
            cbt = const.tile([P, NCB], F32)
            nc.sync.dma_start(cbt[:], cb[:, :])

            # ~20 throwaway matmuls on the const tile warm the PE clock
            # (HAM 4/8 -> 8/8 needs ~3.4us of sustained activity) while the
            # first x block is still in flight
            for wi in range(20):
                wps = pspool.tile([P, NCB], F32, tag="ps", name=f"warm{wi}")
                nc.tensor.matmul(
                    wps[0:NCB, 0:NCB],
                    cbt[:, 0:NCB],
                    cbt[:, 0:NCB],
                    start=True,
                    stop=True,
                )

            def emit_routed_block(xap, w1d, w2d, w2ld, ydram, off, blk,
                                  cb1, cb2, csc1, csc2, w1_pre=None):
                """One token-block of a routed expert job (fp8 DoubleRow)."""
                chunks = _chunks(blk)
                h_t = hrpool.tile([P, FT, blk], FP8, tag="hr")
                for ft in range(FT):
                    if ft == 0 and w1_pre is not None:
                        w1_t = w1_pre
                    else:
                        w1_t = w1rpool.tile([P, KP1, 2, P], FP8, tag="w1r")
                        nc.sync.dma_start(w1_t[:], w1d[ft])
                    pss = [pspool.tile([P, 512], F32, tag="ps", name=f"ps{ci}")
                           for ci in range(len(chunks))]
                    for s in range(KP1):
                        coff = 0
                        for ci, ch in enumerate(chunks):
                            nc.tensor.matmul(
                                pss[ci][:, :ch],
                                w1_t[:, s],
                                xap(s, coff, ch),
                                start=(s == 0),
                                stop=(s == KP1 - 1),
                                perf_mode=DR,
                            )
                            coff += ch
                    coff = 0
                    for ci, ch in enumerate(chunks):
                        nc.scalar.activation(
                            h_t[:, ft, coff:coff + ch],
                            pss[ci][:, :ch],
                            Relu,
                            bias=cbt[:, cb1 + ft:cb1 + ft + 1],
                            scale=cbt[:, csc1:csc1 + 1],
                        )
                        coff += ch

                for dt_i in range(DT):
                    w2_t = w2rpool.tile([P, KP2, 2, P], FP8, tag="w2r")
                    nc.sync.dma_start(w2_t[:], w2d[dt_i])
                    w2l_t = w2rpool.tile([P, P], FP8, tag="w2l")
                    nc.sync.dma_start(w2l_t[:], w2ld[dt_i])
                    y_t = yrpool.tile([P, blk], BF, tag="yr")
                    pss = [pspool.tile([P, 512], F32, tag="ps", name=f"ps{ci}")
                           for ci in range(len(chunks))]
                    for s in range(KP2):
                        coff = 0
                        for ci, ch in enumerate(chunks):
                            nc.tensor.matmul(
                                pss[ci][:, :ch],
                                w2_t[:, s],
                                h_t[:, 2 * s:2 * s + 2, coff:coff + ch],
                                start=(s == 0),
                                stop=False,
                                perf_mode=DR,
                            )
                            coff += ch
                    coff = 0
                    for ci, ch in enumerate(chunks):
                        nc.tensor.matmul(
                            pss[ci][:, :ch],
                            w2l_t[:],
                            h_t[:, FT - 1, coff:coff + ch],
                            start=False,
                            stop=True,
                        )
                        coff += ch
                    coff = 0
                    for ci, ch in enumerate(chunks):
                        nc.scalar.activation(
                            y_t[:, coff:coff + ch],
                            pss[ci][:, :ch],
                            Identity,
                            bias=cbt[:, cb2 + dt_i:cb2 + dt_i + 1],
                            scale=cbt[:, csc2:csc2 + 1],
                        )
                        coff += ch
                    nc.scalar.dma_start(
                        ydram[dt_i, :, off:off + blk], y_t[:, :blk]
                    )

            def emit_routed_blocks(xrpool, xdram, w1d, w2d, w2ld, ydram,
                                   blocks, bmax, cb1, cb2, csc1, csc2,
                                   head=False):
                off = 0
                for bi, blk in enumerate(blocks):
                    w1_first = None
                    if bi == 0 and head:
                        w1_first = [w1rpool.tile([P, KP1, 2, P], FP8,
                                                 tag="w1r", name="w1f")]
                    x_t = xrpool.tile([P, KP1, 2, bmax], FP8, tag="xr")
                    # per-kpair DMA slices run on parallel DMA queues (a
                    # single big DMA lands on one queue at ~115GB/s); for
                    # the head block alternate the two HWDGE rings so issue
                    # pacing doesn't serialize the transfers
                    if w1_first is not None:
                        nc.sync.dma_start(w1_first[0][:], w1d[0])
                        for s in range(KP1):
                            eng = nc.sync if s % 2 == 0 else nc.scalar
                            eng.dma_start(x_t[:, s], xdram[bi, :, s])
                    else:
                        for s in range(KP1):
                            nc.sync.dma_start(x_t[:, s], xdram[bi, :, s])
                    xt_cur = x_t
                    emit_routed_block(
                        lambda s, c0, ch: xt_cur[:, s, :, c0:c0 + ch],
                        w1d, w2d, w2ld, ydram, off, blk,
                        cb1, cb2, csc1, csc2,
                        w1_pre=w1_first[0] if w1_first else None)
                    off += blk

            def emit_shared_job():
                chunks = _chunks(SH_TOK)   # [512]*4
                xs_t = xspool.tile([P, DT, SH_TOK], BF, tag="xs")
                for ko in range(DT):
                    eng = nc.gpsimd if ko % 2 == 0 else nc.scalar
                    eng.dma_start(xs_t[:, ko], xs[:, ko])
                h_t = hspool.tile([P, FT, SH_TOK], BF, tag="hs")
                for ft in range(FT):
                    w1_t = w1spool.tile([P, DT, P], BF, tag="w1s")
                    nc.sync.dma_start(w1_t[:], w1s[ft])
                    pss = [pspool.tile([P, 512], F32, tag="ps", name=f"ps{ci}")
                           for ci in range(len(chunks))]
                    for ko in range(DT):
                        coff = 0
                        for ci, ch in enumerate(chunks):
                            nc.tensor.matmul(
                                pss[ci][:, :ch],
                                w1_t[:, ko],
                                xs_t[:, ko, coff:coff + ch],
                                start=(ko == 0),
                                stop=(ko == DT - 1),
                            )
                            coff += ch
                    coff = 0
                    for ci, ch in enumerate(chunks):
                        nc.scalar.activation(
                            h_t[:, ft, coff:coff + ch],
                            pss[ci][:, :ch],
                            Relu,
                            bias=cbt[:, 0 + ft:0 + ft + 1],
                        )
                        coff += ch

                for dt_i in range(DT):
                    w2_t = w2spool.tile([P, FT, P], BF, tag="w2s")
                    nc.sync.dma_start(w2_t[:], w2s[dt_i])
                    y_t = yrpool.tile([P, SH_TOK], BF, tag="ysh", bufs=1)
                    pss = [pspool.tile([P, 512], F32, tag="ps", name=f"ps{ci}")
                           for ci in range(len(chunks))]
                    for ft in range(FT):
                        coff = 0
                        for ci, ch in enumerate(chunks):
                            nc.tensor.matmul(
                                pss[ci][:, :ch],
                                w2_t[:, ft],
                                h_t[:, ft, coff:coff + ch],
                                start=(ft == 0),
                                stop=(ft == FT - 1),
                            )
                            coff += ch
                    coff = 0
                    for ci, ch in enumerate(chunks):
                        nc.scalar.activation(
                            y_t[:, coff:coff + ch],
                            pss[ci][:, :ch],
                            Identity,
                            bias=cbt[:, 11 + dt_i:11 + dt_i + 1],
                        )
                        coff += ch
                    nc.scalar.dma_start(ys[dt_i, :, :], y_t[:, :SH_TOK])

            with tc.tile_pool(name="xr", bufs=2) as xrpool:
                emit_routed_blocks(xrpool, xra, w1a, w2a, w2al, ya,
                                   blocksA, BA, CB1A, CB2A, CSC, CSC + 1,
                                   head=True)
                emit_shared_job()
                emit_routed_blocks(xrpool, xrb, w1b, w2b, w2bl, yb,
                                   blocksB, BB, CB1B, CB2B, CSC + 2, CSC + 3)

    nc.compile()
    return nc


def _routing(flat, centroids, bias):
    """Replicate the reference router bitwise (jax-CPU sigmoid, stable
    top-4, normalized gates)."""
    import jax
    import jax.numpy as jnp

    cpu = jax.devices("cpu")[0]
    with jax.default_device(cpu):
        scores = np.asarray(
            jax.nn.sigmoid(jnp.asarray(flat) @ jnp.asarray(centroids).T)
            + jnp.asarray(bias)
        )
    idx = np.argsort(-scores, axis=-1, kind="stable")[:, :KR]
    vals = np.take_along_axis(scores, idx, axis=-1)
    gating = vals / np.maximum(vals.sum(-1, keepdims=True, dtype=np.float32), 1e-8)
    return idx.astype(np.int32), gating.astype(np.float32)


def _pow2_scale(w, target=16.0):
    s = float(w.astype(np.float64).std())
    return float(2.0 ** np.round(np.log2(target / max(s, 1e-30))))


def _q8(x, scale):
    return np.clip(x * np.float32(scale), -240.0, 240.0).astype(E4)


def _xr_interleave(x_td):
    """[cnt, D] tokens -> [P, KP1, 2, cnt] fp8 DoubleRow-interleaved, x*S_X."""
    xq = _q8(x_td.T, S_X)                         # [D, cnt]
    return xq.reshape(KP1, 2, P, -1).transpose(2, 0, 1, 3)


def _xr_pack(x_td, blocks):
    """[cnt, D] tokens -> block-major [NB, P, KP1, 2, BMAX] fp8."""
    nb, bmax = max(1, len(blocks)), max(blocks) if blocks else 1
    cnt = x_td.shape[0]
    out = np.zeros((nb, P, KP1, 2, bmax), dtype=E4)
    if cnt:
        xq = _xr_interleave(x_td)
        off = 0
        for bi, blk in enumerate(blocks):
            n = max(0, min(blk, cnt - off))
            if n:
                out[bi, :, :, :, :n] = xq[:, :, :, off:off + n]
            off += blk
    return out


def _xr_head(x_td, hb):
    """First hb tokens -> [P, KP1, 2, hb] fp8 (head block tensor)."""
    cnt = x_td.shape[0]
    out = np.zeros((P, KP1, 2, hb), dtype=E4)
    if cnt:
        n = min(hb, cnt)
        out[:, :, :, :n] = _xr_interleave(x_td[:n])
    return out


def _w1r_tiles(w, scale):
    """[D, F] -> [FT, P, KP1, 2, P] fp8 lhsT tiles (k = 256s+128i+p)."""
    wq = _q8(w, scale)
    return np.ascontiguousarray(
        wq.reshape(KP1, 2, P, FT, P).transpose(3, 2, 0, 1, 4)
    )


def _w2r_tiles(w, scale):
    """[F, D] -> pairs [DT, P, KP2, 2, P] + leftover [DT, P, P] fp8."""
    wq = _q8(w, scale)
    pairs = np.ascontiguousarray(
        wq[: KP2 * 256].reshape(KP2, 2, P, DT, P).transpose(3, 2, 0, 1, 4)
    )
    left = np.ascontiguousarray(
        wq[KP2 * 256:].reshape(P, DT, P).transpose(1, 0, 2)
    )
    return pairs, left


def _xs_tiles(x_td):
    """[T, D] -> [P, DT, T] bf16 feature-major."""
    return np.ascontiguousarray(
        x_td.T.reshape(DT, P, -1).transpose(1, 0, 2)
    ).astype(BF16)


def _w1s_tiles(w):
    """[D, F] -> [FT, P, DT, P] bf16 lhsT tiles."""
    return np.ascontiguousarray(
        w.astype(BF16).reshape(DT, P, FT, P).transpose(2, 1, 0, 3)
    )


def _w2s_tiles(w):
    """[F, D] -> [DT, P, FT, P] bf16 lhsT tiles."""
    return np.ascontiguousarray(
        w.astype(BF16).reshape(FT, P, DT, P).transpose(2, 1, 0, 3)
    )


def kernel(u, shared_w1, shared_b1, shared_w2, shared_b2,
           routed_w1, routed_b1, routed_w2, routed_b2, centroids, bias):
    from concourse.bass_utils import run_bass_kernel_spmd

    _ensure_ntff_hook()
    u = np.asarray(u, dtype=np.float32)
    b, s, d = u.shape
    flat = u.reshape(-1, d)
    T = flat.shape[0]

    idx, gating = _routing(flat, np.asarray(centroids, np.float32),
                           np.asarray(bias, np.float32))

    tok_lists, gate_lists = [], []
    counts = np.zeros(NR, dtype=np.int64)
    for e in range(NR):
        hit = idx == e
        rows = np.nonzero(hit.any(axis=1))[0]
        g = gating[hit].reshape(-1)
        tok_lists.append(rows)
        gate_lists.append(g.astype(np.float32))
        counts[e] = len(rows)

    # big-with-small expert pairing: slot A gets rank i, slot B rank 15-i
    order = np.argsort(-counts, kind="stable")
    slot_a = [int(order[i]) for i in range(NCORES)]
    slot_b = [int(order[NR - 1 - i]) for i in range(NCORES)]
    C1 = max(256, int(max(counts[e] for e in slot_a)))
    C2 = max(256, int(max(counts[e] for e in slot_b)))

    key = (C1, C2)
    if key not in _prog_cache:
        _prog_cache[key] = _build_program(C1, C2)
    nc = _prog_cache[key]

    sw1 = np.asarray(shared_w1, np.float32)
    sb1 = np.asarray(shared_b1, np.float32)
    sw2 = np.asarray(shared_w2, np.float32)
    sb2 = np.asarray(shared_b2, np.float32)
    rw1 = np.asarray(routed_w1, np.float32)
    rb1 = np.asarray(routed_b1, np.float32)
    rw2 = np.asarray(routed_w2, np.float32)
    rb2 = np.asarray(routed_b2, np.float32)

    s_w1 = [_pow2_scale(rw1[e]) for e in range(NR)]
    s_w2 = [_pow2_scale(rw2[e]) for e in range(NR)]
    rw1_t = [_w1r_tiles(rw1[e], s_w1[e]) for e in range(NR)]
    rw2_t = [_w2r_tiles(rw2[e], s_w2[e]) for e in range(NR)]
    sw1_t = [_w1s_tiles(sw1[n]) for n in range(NS)]
    sw2_t = [_w2s_tiles(sw2[n]) for n in range(NS)]

    in_maps = []
    for core in range(NCORES):
        sh_e = core % NS
        sh_off = (core // NS) * SH_TOK
        ea, eb = slot_a[core], slot_b[core]

        cbm = np.zeros((P, 85), np.float32)
        cbm[:, 0:11] = sb1[sh_e].reshape(FT, P).T
        cbm[:, 11:27] = sb2[sh_e].reshape(DT, P).T
        cbm[:, 27:38] = rb1[ea].reshape(FT, P).T * S_H
        cbm[:, 38:54] = rb2[ea].reshape(DT, P).T
        cbm[:, 54:65] = rb1[eb].reshape(FT, P).T * S_H
        cbm[:, 65:81] = rb2[eb].reshape(DT, P).T
        cbm[:, 81] = 1.0 / s_w1[ea]
        cbm[:, 82] = 1.0 / (S_H * s_w2[ea])
        cbm[:, 83] = 1.0 / s_w1[eb]
        cbm[:, 84] = 1.0 / (S_H * s_w2[eb])

        w2a_p, w2a_l = rw2_t[ea]
        w2b_p, w2b_l = rw2_t[eb]
        in_maps.append({
            "xra": _xr_pack(flat[tok_lists[ea]], _blocks(C1)),
            "xrb": _xr_pack(flat[tok_lists[eb]], _blocks(C2)),
            "xs": _xs_tiles(flat[sh_off:sh_off + SH_TOK]),
            "w1a": rw1_t[ea], "w1b": rw1_t[eb],
            "w2a": w2a_p, "w2al": w2a_l,
            "w2b": w2b_p, "w2bl": w2b_l,
            "w1s": sw1_t[sh_e], "w2s": sw2_t[sh_e],
            "cb": cbm,
        })

    res = run_bass_kernel_spmd(nc, in_maps, core_ids=list(range(NCORES)))
    global LAST_RESULT
    LAST_RESULT = res

    out = flat.copy()
    for core in range(NCORES):
        r = res.results[core]
        sh_off = (core // NS) * SH_TOK
        out[sh_off:sh_off + SH_TOK] += (
            r["ys"].astype(np.float32).reshape(D_MODEL, SH_TOK).T
        )
        for nm, e, C in (("ya", slot_a[core], C1), ("yb", slot_b[core], C2)):
            rows = tok_lists[e]
            if len(rows):
                ye = (r[nm].astype(np.float32)
                      .reshape(D_MODEL, C)[:, :len(rows)].T)
                out[rows] += gate_lists[e][:, None] * ye

    return out.reshape(b, s, d)
